# revision 15
# baseline (speedup 1.0000x reference)
"""BERT self-attention (B=4, S=2048, D=1024, H=16) on 8 trn2 NeuronCores.

Sharding: core c -> (batch b = c//2, head-group hg = c%2, 8 heads each).
Each core computes out[b, :, hg*512:(hg+1)*512] independently; host
gathers. Inputs are pre-transposed AND pre-tiled on host so every DMA is
contiguous >=4KB per partition: xt = X.T [D,S]; weights are partition-
major ([128, dt, ...] with row p holding W.T[dt*128+p, cols]).

v5 design (all-bf16, fully software-pipelined, ACT(exp)-paced):
  - V-projection raced against the xt DMA: dt-outer over ALL 8 PSUM
    banks (2 passes x 8 s-tiles); pass drains interleaved into the last
    dt row so DVE never serializes the pass handoff.
  - QK projection: kt(p0)+qt(p0,c0) up front; every remaining chunk
    drip-fed into attention jt-slots (PE slack while ACT runs exp),
    paced so pair p+1's Q/K finish during pair p, skipping the first 2
    slots of each qc (boundary pressure).
  - Attention per (pair, qc): scores for 2 heads as concurrent
    row-group matmuls -> [128,2,512] PSUM; ONE exp per jt ([128,1024]
    ACT op, mask as bias); ctx accumulates in PSUM and LAGS TWO SLOTS
    (sp bufs=2, u bufs=3) so PE never head-of-line blocks on exp and
    the C-bank drain gets slack before ctx(start=True) reuses it.
  - Drain: C -> SBUF copy (split per head, overlapped with last ctx),
    then 8 (transpose, reciprocal, scale) steps in slots 4..11 of the
    next qc; one batched DMA out per (p,qc).  Final drain fans out over
    4 free PSUM banks.
PSUM: sp0(2) + sp1(2) + c0(1) + c1(1) + proj(1) + tp(1) = 8 banks.
"""

import ml_dtypes
import numpy as np

import concourse.bass as bass
import concourse.tile as tile
from concourse import bacc, mybir
from concourse.bass_utils import run_bass_kernel_spmd
from concourse.masks import make_identity

B, S, D, H = 4, 2048, 1024, 16
DH = 64
O = 512  # per-core output width (8 heads)
HL = 8  # local heads per core
NP = 4  # head pairs per core
ST = S // 128  # 16 s-tiles
QC = 4  # query quarters (512 queries each)
F32 = mybir.dt.float32
BF16 = mybir.dt.bfloat16
EXP = mybir.ActivationFunctionType.Exp

_NC_CACHE = None


def build_nc():
    nc = bacc.Bacc(
        "TRN2",
        target_bir_lowering=False,
        debug=False,
        enable_asserts=True,
        num_devices=8,
    )
    xt = nc.dram_tensor("xt", [D, S], BF16, kind="ExternalInput").ap()
    # partition-major pre-tiled weights (see _make_in_maps)
    wqt = nc.dram_tensor("wqt", [128, NP, 8, 128], BF16, kind="ExternalInput").ap()
    wkt = nc.dram_tensor("wkt", [128, NP, 8, 128], BF16, kind="ExternalInput").ap()
    wvt = nc.dram_tensor("wvt", [128, 8, O], BF16, kind="ExternalInput").ap()
    bq = nc.dram_tensor("bq", [O], F32, kind="ExternalInput").ap()
    bk = nc.dram_tensor("bk", [O], F32, kind="ExternalInput").ap()
    bv = nc.dram_tensor("bv", [O], F32, kind="ExternalInput").ap()
    mask = nc.dram_tensor("mask", [S], F32, kind="ExternalInput").ap()
    out = nc.dram_tensor("out", [S, O], F32, kind="ExternalOutput").ap()

    with tile.TileContext(nc) as tc:
        _emit(nc, tc, xt, wqt, wkt, wvt, bq, bk, bv, mask, out)
    nc.compile()
    return nc


def _emit(nc, tc, xt, wqt, wkt, wvt, bq, bk, bv, mask, out):
    with (
        tc.tile_pool(name="singles", bufs=1) as singles,
        tc.tile_pool(name="persist", bufs=1) as persist,
        tc.tile_pool(name="wpool", bufs=1) as wpool,
        tc.tile_pool(name="attn", bufs=1) as attn,
        tc.tile_pool(name="psum", bufs=1, space="PSUM") as psum,
    ):
        # persistent activations (all bf16)
        xts = [persist.tile([128, S], BF16, name=f"xts{dt}", tag=f"xts{dt}") for dt in range(8)]
        qts = [persist.tile([128, S], BF16, name=f"qt{p}", tag=f"qt{p}") for p in range(NP)]
        kts = [persist.tile([128, S], BF16, name=f"kt{p}", tag=f"kt{p}") for p in range(NP)]
        vaug = [
            persist.tile([128, HL, DH + 1], BF16, name=f"vaug{t}", tag=f"vaug{t}")
            for t in range(ST)
        ]

        # DMA order is the startup critical path: wv first (V proj races
        # the xt stream), then xt, then small/late-needed tensors, then
        # QK weights (first used ~35us in).
        wv = wpool.tile([128, 8, O], BF16, name="wv", tag="wv")
        nc.sync.dma_start(out=wv[:, 0:1, :], in_=wvt[:, 0:1, :])
        nc.sync.dma_start(out=xts[0], in_=xt[0:128, :])
        nc.sync.dma_start(out=wv[:, 1:4, :], in_=wvt[:, 1:4, :])
        bv_bc = singles.tile([128, HL, DH], F32)
        nc.sync.dma_start(
            out=bv_bc, in_=bass.AP(tensor=bv.tensor, offset=0, ap=[[0, 128], [1, O]])
        )
        nc.sync.dma_start(out=xts[1], in_=xt[128:256, :])
        nc.sync.dma_start(out=wv[:, 4:8, :], in_=wvt[:, 4:8, :])
        for dt in range(2, 8):
            nc.sync.dma_start(out=xts[dt], in_=xt[dt * 128 : (dt + 1) * 128, :])
        ident_bf = singles.tile([128, 128], BF16)
        make_identity(nc, ident_bf)
        wk = wpool.tile([128, NP, 8, 128], BF16, name="wk", tag="wk")
        nc.sync.dma_start(out=wk, in_=wkt)
        wq = wpool.tile([128, NP, 8, 128], BF16, name="wq", tag="wq")
        nc.sync.dma_start(out=wq, in_=wqt)
        wsl = {"k": wk, "q": wq}
        bq_sb = singles.tile([128, NP], F32)
        nc.sync.dma_start(out=bq_sb, in_=bq.rearrange("(t p) -> p t", p=128))
        bk_sb = singles.tile([128, NP], F32)
        nc.sync.dma_start(out=bk_sb, in_=bk.rearrange("(t p) -> p t", p=128))
        mask_sb = singles.tile([128, ST], F32)
        nc.sync.dma_start(out=mask_sb, in_=mask.rearrange("(t p) -> p t", p=128))
        # Schraudolph fast-exp constants for the DVE-offloaded slots:
        # exp(0.125*s + m) ~= bitcast_f32(int32(s*SCH_A + (m*SCH_M + SCH_B)))
        # with SCH_B tuned for minimax relative error (~+-3.5%).
        SCH_A = 0.125 * 1.4426950408889634 * 8388608.0
        SCH_M = 1.4426950408889634 * 8388608.0
        SCH_B = 127.0 * 8388608.0 - 297795.0
        mb_sb = singles.tile([128, ST], F32)
        nc.vector.tensor_scalar(
            out=mb_sb, in0=mask_sb, scalar1=SCH_M, scalar2=SCH_B,
            op0=mybir.AluOpType.mult, op1=mybir.AluOpType.add,
        )

        # vaug ones-columns: DVE is idle now, do them all up front
        for st in range(ST):
            nc.vector.memset(vaug[st][:, :, DH : DH + 1], 1.0)

        # ---- V projection: 2 passes x 8 s-tiles over all 8 PSUM banks,
        # dt-outer so pass 1 consumes xt chunks as the DMA delivers them.
        # Drains interleave into the dt=7 row so the pass handoff never
        # serializes on DVE.
        def v_pass(sb):  # sb = base s-tile (0 or 8)
            t01 = psum.tile([128, 2, HL, DH], F32, name=f"psv{sb}a", tag="sp0", bufs=1)
            t23 = psum.tile([128, 2, HL, DH], F32, name=f"psv{sb}b", tag="sp1", bufs=1)
            singles_ = [
                psum.tile([128, HL, DH], F32, name=f"psv{sb}_{i}", tag=t, bufs=1)
                for i, t in enumerate(("c0", "c1", "proj", "tp"))
            ]
            dsts = [t01[:, 0], t01[:, 1], t23[:, 0], t23[:, 1]] + singles_
            for dt in range(8):
                for g in range(8):
                    st = sb + g
                    nc.tensor.matmul(
                        dsts[g],
                        xts[dt][:, st * 128 : (st + 1) * 128],
                        wv[:, dt, :],
                        start=(dt == 0),
                        stop=(dt == 7),
                    )
                    if dt == 7:
                        va = vaug[st]
                        nc.vector.tensor_add(va[:, :, 0:DH], dsts[g], bv_bc)

        v_pass(0)
        v_pass(8)

        # ---- QK projection machinery ----
        qk_tag = [0]

        def emit_qk_chunk(which, p, c):
            w = wsl[which]
            dst = {"k": kts, "q": qts}[which][p]
            bias_sb = {"k": bk_sb, "q": bq_sb}[which]
            tag = ("proj", "tp")[qk_tag[0] % 2]
            qk_tag[0] += 1
            ps = psum.tile([128, 512], F32, name=f"ps{which}{p}_{c}", tag=tag, bufs=1)
            for dt in range(8):
                nc.tensor.matmul(
                    ps,
                    w[:, p, dt, :],
                    xts[dt][:, c * 512 : (c + 1) * 512],
                    start=(dt == 0),
                    stop=(dt == 7),
                )
            nc.vector.tensor_scalar_add(
                dst[:, c * 512 : (c + 1) * 512], ps, bias_sb[:, p : p + 1]
            )

        # Upfront: all of kt(p0) (scores at (p0,qc0) span every key chunk)
        # plus qt(p0,c0).
        for c in range(4):
            emit_qk_chunk("k", 0, c)
        emit_qk_chunk("q", 0, 0)

        # Remaining chunks drip-fed into attention slots.
        proj_chunks = [("q", 0, 1), ("q", 0, 2), ("q", 0, 3)]
        for p in range(1, NP):
            for c in range(4):
                proj_chunks.append(("k", p, c))
            for c in range(4):
                proj_chunks.append(("q", p, c))
        proj_cursor = [0, 0]  # chunk index, dt index

        def proj_mms_left():
            ci, dt = proj_cursor
            return (len(proj_chunks) - ci) * 8 - dt

        def emit_proj_mm():
            ci, dt = proj_cursor
            if ci >= len(proj_chunks):
                return False
            which, p, c = proj_chunks[ci]
            if dt == 0:
                emit_proj_mm.ps = psum.tile(
                    [128, 512], F32, name=f"ps{which}{p}_{c}", tag="proj", bufs=1
                )
            nc.tensor.matmul(
                emit_proj_mm.ps,
                wsl[which][:, p, dt, :],
                xts[dt][:, c * 512 : (c + 1) * 512],
                start=(dt == 0),
                stop=(dt == 7),
            )
            if dt == 7:
                dst = {"k": kts, "q": qts}[which][p]
                bias_sb = {"k": bk_sb, "q": bq_sb}[which]
                nc.vector.tensor_scalar_add(
                    dst[:, c * 512 : (c + 1) * 512], emit_proj_mm.ps,
                    bias_sb[:, p : p + 1],
                )
                proj_cursor[0] += 1
                proj_cursor[1] = 0
            else:
                proj_cursor[1] += 1
            return True

        # ---- attention: global software pipeline over 256 (p,qc,jt)
        # slots.  Slot g: scores(g) -> exp(g) -> [proj drip] -> ctx(g-2)
        # -> [drain step in slots 4..11].
        steps = [(p, qc, jt) for p in range(NP) for qc in range(QC) for jt in range(ST)]
        hist = {}
        Cs = {}
        drain_q = []

        def emit_scores(g, p, qc, jt):
            base = qc * 512
            sp = psum.tile([128, 2, 512], F32, name=f"sp{g}", tag=f"sp{g % 2}", bufs=1)
            for x in range(2):
                hp = slice(x * 64, x * 64 + 64)
                nc.tensor.matmul(
                    sp[:, x, :],
                    kts[p][hp, jt * 128 : (jt + 1) * 128],
                    qts[p][hp, base : base + 512],
                    start=True,
                    stop=True,
                )
            u = attn.tile([128, 2, 512], BF16, name=f"u{g}", tag=f"u{g % 4}", bufs=1)
            if jt in (5, 13):
                # offload this slot's exp to the (otherwise idle) DVE via
                # Schraudolph's bit-trick so ACT catches up to the PE pace.
                iu = attn.tile(
                    [128, 2, 512], mybir.dt.int32, name=f"iu{g}", tag=f"iu{g % 2}",
                    bufs=1,
                )
                nc.vector.tensor_scalar(
                    out=iu, in0=sp, scalar1=SCH_A, scalar2=mb_sb[:, jt : jt + 1],
                    op0=mybir.AluOpType.mult, op1=mybir.AluOpType.add,
                )
                nc.vector.tensor_copy(out=u, in_=iu[:, :, :].bitcast(F32))
            else:
                nc.scalar.activation(
                    u, sp, EXP, bias=mask_sb[:, jt : jt + 1], scale=0.125
                )
            hist[g] = (u, p, qc, jt)

        def emit_ctx(g):
            u, p, qc, jt = hist.pop(g)
            if jt == 0:
                Cs[p, qc] = [
                    psum.tile(
                        [DH + 1, 512], F32, name=f"c{x}_{p}_{qc}", tag=f"c{x}", bufs=1
                    )
                    for x in range(2)
                ]
            C = Cs[p, qc]
            last = jt == ST - 1
            csb = []
            for x in range(2):
                nc.tensor.matmul(
                    C[x],
                    vaug[jt][:, 2 * p + x, :],
                    u[:, x, :],
                    start=(jt == 0),
                    stop=last,
                )
                if last:
                    # bf16 staging, 80 rows (mult of 16) so the XBAR DMA
                    # transpose is legal; rows 65..79 are never read back.
                    cs = attn.tile(
                        [80, 512], BF16, name=f"csb{p}_{qc}_{x}", tag=f"csb{x}",
                        bufs=2,
                    )
                    nc.vector.tensor_copy(out=cs[0 : DH + 1, :], in_=C[x])
                    csb.append(cs)
            if last:
                Cs.pop((p, qc))
                ot = attn.tile(
                    [128, 4, 2, DH], F32, name=f"ot{p}_{qc}", tag="ot", bufs=2
                )
                drain_q.append(
                    {"csb": csb, "ot": ot, "p": p, "qc": qc, "i": 0, "xbar": True}
                )

        def emit_drain_step(tag="tp"):
            if not drain_q:
                return
            dr = drain_q[0]
            i = dr["i"]
            it, x = i // 2, i % 2
            if dr.get("xbar"):
                # transpose via the DMA XBAR (SBUF->SBUF, 2-byte) to keep
                # the PE free for matmuls
                tp_ = attn.tile(
                    [128, 80], BF16, name=f"tpx{dr['p']}_{dr['qc']}_{i}",
                    tag=f"tpx{i % 2}", bufs=1,
                )
                nc.sync.dma_start(
                    out=tp_,
                    in_=dr["csb"][x][:, it * 128 : (it + 1) * 128],
                    transpose=True,
                )
            else:
                tp_ = psum.tile(
                    [128, DH + 1], BF16, name=f"tp{dr['p']}_{dr['qc']}_{i}",
                    tag=tag, bufs=1,
                )
                nc.tensor.transpose(
                    tp_,
                    dr["csb"][x][0 : DH + 1, it * 128 : (it + 1) * 128],
                    ident_bf[0 : DH + 1, 0 : DH + 1],
                )
            rc = attn.tile(
                [128, 1], F32, name=f"rc{dr['p']}_{dr['qc']}_{i}", tag="rc", bufs=4
            )
            nc.vector.reciprocal(rc, tp_[:, DH : DH + 1])
            nc.vector.tensor_scalar_mul(dr["ot"][:, it, x, :], tp_[:, 0:DH], rc)
            dr["i"] += 1
            if dr["i"] == 8:
                p, qc = dr["p"], dr["qc"]
                base = qc * 512
                nc.sync.dma_start(
                    out=out[
                        base : base + 512, 2 * p * DH : (2 * p + 2) * DH
                    ].rearrange("(i p) c -> p i c", p=128),
                    in_=dr["ot"],
                )
                drain_q.pop(0)

        NSLOT = len(steps)
        for g, (p, qc, jt) in enumerate(steps):
            emit_scores(g, p, qc, jt)
            left = proj_mms_left()
            if left > 0 and jt >= 2:
                quota = min(2, max(0, -(-left // max(1, (NSLOT - g)))))
                ci = proj_cursor[0]
                need_this_pair = 0
                for k in range(ci, len(proj_chunks)):
                    if proj_chunks[k][1] <= p + 1:
                        need_this_pair += 8
                    else:
                        break
                if need_this_pair:
                    need_this_pair -= proj_cursor[1]
                    pair_slots_left = 64 - (g % 64)
                    quota = min(2, max(quota, -(-need_this_pair // pair_slots_left)))
                for _ in range(quota):
                    emit_proj_mm()
            if g >= 3:
                emit_ctx(g - 3)
            if 4 <= jt <= 11:
                emit_drain_step()
        # pipeline tail: last three ctx slots, then the final drain fanned
        # out over the 4 now-free PSUM banks.
        emit_ctx(NSLOT - 3)
        emit_ctx(NSLOT - 2)
        emit_ctx(NSLOT - 1)
        # final drain: PE transposes over the 4 now-free PSUM banks (the
        # XBAR round-trip latency would stretch the tail)
        k = 0
        while drain_q:
            drain_q[0]["xbar"] = False
            emit_drain_step(tag=("proj", "tp", "c0", "c1")[k % 4])
            k += 1


def _make_in_maps(hidden_states, attention_mask, Wq, bq, Wk, bk, Wv, bv):
    bf = ml_dtypes.bfloat16

    def wqk_tiled(W, sl):
        # W.T shard [D, O] -> [128, NP, 8, 128]: row p holds
        # W.T[dt*128+p, pair*128:(pair+1)*128] at [p, pair, dt, :]
        wt = W[sl, :].T.astype(bf)  # [D, O]
        return np.ascontiguousarray(
            wt.reshape(8, 128, NP, 128).transpose(1, 2, 0, 3)
        )

    def wv_tiled(W, sl):
        wt = W[sl, :].T.astype(bf)  # [D, O]
        return np.ascontiguousarray(wt.reshape(8, 128, O).transpose(1, 0, 2))

    in_maps = []
    for c in range(8):
        b, hg = divmod(c, 2)
        sl = slice(hg * O, (hg + 1) * O)
        in_maps.append(
            {
                "xt": np.ascontiguousarray(hidden_states[b].T.astype(bf)),
                "wqt": wqk_tiled(Wq, sl),
                "wkt": wqk_tiled(Wk, sl),
                "wvt": wv_tiled(Wv, sl),
                "bq": np.ascontiguousarray(bq[sl]),
                "bk": np.ascontiguousarray(bk[sl]),
                "bv": np.ascontiguousarray(bv[sl]),
                "mask": np.ascontiguousarray(attention_mask[b, 0, 0, :]),
            }
        )
    return in_maps


def _gather(results):
    out = np.empty((B, S, D), dtype=np.float32)
    for c in range(8):
        b, hg = divmod(c, 2)
        out[b, :, hg * O : (hg + 1) * O] = results[c]["out"]
    return out


def kernel(hidden_states, attention_mask, Wq, bq, Wk, bk, Wv, bv, **run_kwargs):
    global _NC_CACHE
    args = [hidden_states, attention_mask, Wq, bq, Wk, bk, Wv, bv]
    args = [np.asarray(a, dtype=np.float32) for a in args]
    if _NC_CACHE is None:
        _NC_CACHE = build_nc()
    in_maps = _make_in_maps(*args)
    res = run_bass_kernel_spmd(_NC_CACHE, in_maps, core_ids=list(range(8)), **run_kwargs)
    kernel.last_result = res
    return _gather(res.results)


# revision 17
# speedup vs baseline: 1.0883x; 1.0883x over previous
"""BERT self-attention (B=4, S=2048, D=1024, H=16) on 8 trn2 NeuronCores.

Sharding: core c -> (batch b = c//2, head-group hg = c%2, 8 heads each).
Each core computes out[b, :, hg*512:(hg+1)*512] independently; host
gathers. Inputs are pre-transposed AND pre-tiled on host so every DMA is
contiguous >=4KB per partition: xt = X.T [D,S]; weights are partition-
major ([128, dt, ...] with row p holding W.T[dt*128+p, cols]).

v5 design (all-bf16, fully software-pipelined, ACT(exp)-paced):
  - V-projection raced against the xt DMA: dt-outer over ALL 8 PSUM
    banks (2 passes x 8 s-tiles); pass drains interleaved into the last
    dt row so DVE never serializes the pass handoff.
  - QK projection: kt(p0)+qt(p0,c0) up front; every remaining chunk
    drip-fed into attention jt-slots (PE slack while ACT runs exp),
    paced so pair p+1's Q/K finish during pair p, skipping the first 2
    slots of each qc (boundary pressure).
  - Attention per (pair, qc): scores for 2 heads as concurrent
    row-group matmuls -> [128,2,512] PSUM; ONE exp per jt ([128,1024]
    ACT op, mask as bias); ctx accumulates in PSUM and LAGS TWO SLOTS
    (sp bufs=2, u bufs=3) so PE never head-of-line blocks on exp and
    the C-bank drain gets slack before ctx(start=True) reuses it.
  - Drain: C -> SBUF copy (split per head, overlapped with last ctx),
    then 8 (transpose, reciprocal, scale) steps in slots 4..11 of the
    next qc; one batched DMA out per (p,qc).  Final drain fans out over
    4 free PSUM banks.
PSUM: sp0(2) + sp1(2) + c0(1) + c1(1) + proj(1) + tp(1) = 8 banks.
"""

import ml_dtypes
import numpy as np

import concourse.bass as bass
import concourse.tile as tile
from concourse import bacc, mybir
from concourse.bass_utils import run_bass_kernel_spmd
from concourse.masks import make_identity

B, S, D, H = 4, 2048, 1024, 16
DH = 64
O = 512  # per-core output width (8 heads)
HL = 8  # local heads per core
NP = 4  # head pairs per core
ST = S // 128  # 16 s-tiles
QC = 4  # query quarters (512 queries each)
F32 = mybir.dt.float32
BF16 = mybir.dt.bfloat16
EXP = mybir.ActivationFunctionType.Exp

_NC_CACHE = None


def build_nc():
    nc = bacc.Bacc(
        "TRN2",
        target_bir_lowering=False,
        debug=False,
        enable_asserts=True,
        num_devices=8,
    )
    xt = nc.dram_tensor("xt", [D, S], BF16, kind="ExternalInput").ap()
    # partition-major pre-tiled weights (see _make_in_maps)
    wqt = nc.dram_tensor("wqt", [128, NP, 8, 128], BF16, kind="ExternalInput").ap()
    wkt = nc.dram_tensor("wkt", [128, NP, 8, 128], BF16, kind="ExternalInput").ap()
    wvt = nc.dram_tensor("wvt", [128, 8, O], BF16, kind="ExternalInput").ap()
    bq = nc.dram_tensor("bq", [O], F32, kind="ExternalInput").ap()
    bk = nc.dram_tensor("bk", [O], F32, kind="ExternalInput").ap()
    bv = nc.dram_tensor("bv", [O], F32, kind="ExternalInput").ap()
    mask = nc.dram_tensor("mask", [S], F32, kind="ExternalInput").ap()
    out = nc.dram_tensor("out", [S, O], F32, kind="ExternalOutput").ap()

    with tile.TileContext(nc) as tc:
        _emit(nc, tc, xt, wqt, wkt, wvt, bq, bk, bv, mask, out)
    nc.compile()
    return nc


def _emit(nc, tc, xt, wqt, wkt, wvt, bq, bk, bv, mask, out):
    with (
        tc.tile_pool(name="singles", bufs=1) as singles,
        tc.tile_pool(name="persist", bufs=1) as persist,
        tc.tile_pool(name="wpool", bufs=1) as wpool,
        tc.tile_pool(name="attn", bufs=1) as attn,
        tc.tile_pool(name="psum", bufs=1, space="PSUM") as psum,
    ):
        # persistent activations (all bf16)
        xts = [persist.tile([128, S], BF16, name=f"xts{dt}", tag=f"xts{dt}") for dt in range(8)]
        qts = [persist.tile([128, S], BF16, name=f"qt{p}", tag=f"qt{p}") for p in range(NP)]
        kts = [persist.tile([128, S], BF16, name=f"kt{p}", tag=f"kt{p}") for p in range(NP)]
        vaug = [
            persist.tile([128, HL, DH + 1], BF16, name=f"vaug{t}", tag=f"vaug{t}")
            for t in range(ST)
        ]

        # DMA order is the startup critical path: wv first (V proj races
        # the xt stream), then xt, then small/late-needed tensors, then
        # QK weights (first used ~35us in).
        wv = wpool.tile([128, 8, O], BF16, name="wv", tag="wv")
        nc.sync.dma_start(out=wv[:, 0:1, :], in_=wvt[:, 0:1, :])
        nc.sync.dma_start(out=xts[0], in_=xt[0:128, :])
        nc.sync.dma_start(out=wv[:, 1:4, :], in_=wvt[:, 1:4, :])
        bv_bc = singles.tile([128, HL, DH], F32)
        nc.sync.dma_start(
            out=bv_bc, in_=bass.AP(tensor=bv.tensor, offset=0, ap=[[0, 128], [1, O]])
        )
        nc.sync.dma_start(out=xts[1], in_=xt[128:256, :])
        nc.sync.dma_start(out=wv[:, 4:8, :], in_=wvt[:, 4:8, :])
        for dt in range(2, 8):
            nc.sync.dma_start(out=xts[dt], in_=xt[dt * 128 : (dt + 1) * 128, :])
        ident = singles.tile([128, 128], F32)
        make_identity(nc, ident)
        wk = wpool.tile([128, NP, 8, 128], BF16, name="wk", tag="wk")
        nc.sync.dma_start(out=wk, in_=wkt)
        wq = wpool.tile([128, NP, 8, 128], BF16, name="wq", tag="wq")
        nc.sync.dma_start(out=wq, in_=wqt)
        wsl = {"k": wk, "q": wq}
        bq_sb = singles.tile([128, NP], F32)
        nc.sync.dma_start(out=bq_sb, in_=bq.rearrange("(t p) -> p t", p=128))
        bk_sb = singles.tile([128, NP], F32)
        nc.sync.dma_start(out=bk_sb, in_=bk.rearrange("(t p) -> p t", p=128))
        mask_sb = singles.tile([128, ST], F32)
        nc.sync.dma_start(out=mask_sb, in_=mask.rearrange("(t p) -> p t", p=128))
        # Schraudolph fast-exp constants for the DVE-offloaded slots:
        # exp(0.125*s + m) ~= bitcast_f32(int32(s*SCH_A + (m*SCH_M + SCH_B)))
        # with SCH_B tuned for minimax relative error (~+-3.5%).
        SCH_A = 0.125 * 1.4426950408889634 * 8388608.0
        SCH_M = 1.4426950408889634 * 8388608.0
        SCH_B = 127.0 * 8388608.0 - 297795.0
        mb_sb = singles.tile([128, ST], F32)
        nc.vector.tensor_scalar(
            out=mb_sb, in0=mask_sb, scalar1=SCH_M, scalar2=SCH_B,
            op0=mybir.AluOpType.mult, op1=mybir.AluOpType.add,
        )

        # vaug ones-columns: DVE is idle now, do them all up front
        for st in range(ST):
            nc.vector.memset(vaug[st][:, :, DH : DH + 1], 1.0)

        # ---- V projection: 2 passes x 8 s-tiles over all 8 PSUM banks,
        # dt-outer so pass 1 consumes xt chunks as the DMA delivers them.
        # Drains interleave into the dt=7 row so the pass handoff never
        # serializes on DVE.
        def v_pass(sb):  # sb = base s-tile (0 or 8)
            t01 = psum.tile([128, 2, HL, DH], F32, name=f"psv{sb}a", tag="sp0", bufs=1)
            t23 = psum.tile([128, 2, HL, DH], F32, name=f"psv{sb}b", tag="sp1", bufs=1)
            singles_ = [
                psum.tile([128, HL, DH], F32, name=f"psv{sb}_{i}", tag=t, bufs=1)
                for i, t in enumerate(("c0", "c1", "proj", "tp"))
            ]
            dsts = [t01[:, 0], t01[:, 1], t23[:, 0], t23[:, 1]] + singles_
            for dt in range(8):
                for g in range(8):
                    st = sb + g
                    nc.tensor.matmul(
                        dsts[g],
                        xts[dt][:, st * 128 : (st + 1) * 128],
                        wv[:, dt, :],
                        start=(dt == 0),
                        stop=(dt == 7),
                    )
                    if dt == 7:
                        va = vaug[st]
                        nc.vector.tensor_add(va[:, :, 0:DH], dsts[g], bv_bc)

        v_pass(0)
        v_pass(8)

        # ---- QK projection machinery ----
        qk_tag = [0]

        def emit_qk_chunk(which, p, c):
            w = wsl[which]
            dst = {"k": kts, "q": qts}[which][p]
            bias_sb = {"k": bk_sb, "q": bq_sb}[which]
            tag = ("proj", "tp")[qk_tag[0] % 2]
            qk_tag[0] += 1
            ps = psum.tile([128, 512], F32, name=f"ps{which}{p}_{c}", tag=tag, bufs=1)
            for dt in range(8):
                nc.tensor.matmul(
                    ps,
                    w[:, p, dt, :],
                    xts[dt][:, c * 512 : (c + 1) * 512],
                    start=(dt == 0),
                    stop=(dt == 7),
                )
            nc.vector.tensor_scalar_add(
                dst[:, c * 512 : (c + 1) * 512], ps, bias_sb[:, p : p + 1]
            )

        # Upfront: only what gates attention slot 0 (kt c0 covers keys for
        # jt 0..3, qt c0 the first query quarter) plus kt c1 (jt4, too
        # soon for the drip).  kt c2/c3 are needed at jt8/jt12 and are
        # dripped.
        emit_qk_chunk("k", 0, 0)
        emit_qk_chunk("q", 0, 0)
        emit_qk_chunk("k", 0, 1)

        # Remaining chunks drip-fed into attention slots.
        proj_chunks = [("k", 0, 2), ("k", 0, 3), ("q", 0, 1), ("q", 0, 2), ("q", 0, 3)]
        for p in range(1, NP):
            for c in range(4):
                proj_chunks.append(("k", p, c))
            for c in range(4):
                proj_chunks.append(("q", p, c))
        proj_cursor = [0, 0]  # chunk index, dt index

        def proj_mms_left():
            ci, dt = proj_cursor
            return (len(proj_chunks) - ci) * 8 - dt

        def emit_proj_mm():
            ci, dt = proj_cursor
            if ci >= len(proj_chunks):
                return False
            which, p, c = proj_chunks[ci]
            if dt == 0:
                emit_proj_mm.ps = psum.tile(
                    [128, 512], F32, name=f"ps{which}{p}_{c}", tag="proj", bufs=1
                )
            nc.tensor.matmul(
                emit_proj_mm.ps,
                wsl[which][:, p, dt, :],
                xts[dt][:, c * 512 : (c + 1) * 512],
                start=(dt == 0),
                stop=(dt == 7),
            )
            if dt == 7:
                dst = {"k": kts, "q": qts}[which][p]
                bias_sb = {"k": bk_sb, "q": bq_sb}[which]
                nc.vector.tensor_scalar_add(
                    dst[:, c * 512 : (c + 1) * 512], emit_proj_mm.ps,
                    bias_sb[:, p : p + 1],
                )
                proj_cursor[0] += 1
                proj_cursor[1] = 0
            else:
                proj_cursor[1] += 1
            return True

        # ---- attention: global software pipeline over 256 (p,qc,jt)
        # slots.  Slot g: scores(g) -> exp(g) -> [proj drip] -> ctx(g-2)
        # -> [drain step in slots 4..11].
        steps = [(p, qc, jt) for p in range(NP) for qc in range(QC) for jt in range(ST)]
        hist = {}
        Cs = {}
        drain_q = []

        def emit_scores(g, p, qc, jt):
            base = qc * 512
            sp = psum.tile([128, 2, 512], F32, name=f"sp{g}", tag=f"sp{g % 2}", bufs=1)
            for x in range(2):
                hp = slice(x * 64, x * 64 + 64)
                nc.tensor.matmul(
                    sp[:, x, :],
                    kts[p][hp, jt * 128 : (jt + 1) * 128],
                    qts[p][hp, base : base + 512],
                    start=True,
                    stop=True,
                )
            u = attn.tile([128, 2, 512], BF16, name=f"u{g}", tag=f"u{g % 4}", bufs=1)
            if jt == 13:
                # offload this slot's exp to the (otherwise idle) DVE via
                # Schraudolph's bit-trick so ACT catches up to the PE pace.
                iu = attn.tile(
                    [128, 2, 512], mybir.dt.int32, name=f"iu{g}", tag=f"iu{g % 2}",
                    bufs=1,
                )
                nc.vector.tensor_scalar(
                    out=iu, in0=sp, scalar1=SCH_A, scalar2=mb_sb[:, jt : jt + 1],
                    op0=mybir.AluOpType.mult, op1=mybir.AluOpType.add,
                )
                nc.vector.tensor_copy(out=u, in_=iu[:, :, :].bitcast(F32))
            else:
                nc.scalar.activation(
                    u, sp, EXP, bias=mask_sb[:, jt : jt + 1], scale=0.125
                )
            hist[g] = (u, p, qc, jt)

        def emit_ctx(g):
            u, p, qc, jt = hist.pop(g)
            if jt == 0:
                Cs[p, qc] = [
                    psum.tile(
                        [DH + 1, 512], F32, name=f"c{x}_{p}_{qc}", tag=f"c{x}", bufs=1
                    )
                    for x in range(2)
                ]
            C = Cs[p, qc]
            last = jt == ST - 1
            csb = []
            for x in range(2):
                nc.tensor.matmul(
                    C[x],
                    vaug[jt][:, 2 * p + x, :],
                    u[:, x, :],
                    start=(jt == 0),
                    stop=last,
                )
                if last:
                    cs = attn.tile(
                        [DH + 1, 512], F32, name=f"csb{p}_{qc}_{x}", tag=f"csb{x}",
                        bufs=2,
                    )
                    nc.vector.tensor_copy(out=cs, in_=C[x])
                    csb.append(cs)
            if last:
                Cs.pop((p, qc))
                ot = attn.tile(
                    [128, 4, 2, DH], F32, name=f"ot{p}_{qc}", tag="ot", bufs=2
                )
                drain_q.append({"csb": csb, "ot": ot, "p": p, "qc": qc, "i": 0})

        def emit_drain_step(tag="tp"):
            if not drain_q:
                return
            dr = drain_q[0]
            i = dr["i"]
            it, x = i // 2, i % 2
            tp_ = psum.tile(
                [128, DH + 1], F32, name=f"tp{dr['p']}_{dr['qc']}_{i}", tag=tag, bufs=1
            )
            nc.tensor.transpose(
                tp_,
                dr["csb"][x][:, it * 128 : (it + 1) * 128],
                ident[0 : DH + 1, 0 : DH + 1],
            )
            rc = attn.tile(
                [128, 1], F32, name=f"rc{dr['p']}_{dr['qc']}_{i}", tag="rc", bufs=4
            )
            nc.vector.reciprocal(rc, tp_[:, DH : DH + 1])
            nc.vector.tensor_scalar_mul(dr["ot"][:, it, x, :], tp_[:, 0:DH], rc)
            dr["i"] += 1
            if dr["i"] == 8:
                p, qc = dr["p"], dr["qc"]
                base = qc * 512
                nc.sync.dma_start(
                    out=out[
                        base : base + 512, 2 * p * DH : (2 * p + 2) * DH
                    ].rearrange("(i p) c -> p i c", p=128),
                    in_=dr["ot"],
                )
                drain_q.pop(0)

        NSLOT = len(steps)
        for g, (p, qc, jt) in enumerate(steps):
            emit_scores(g, p, qc, jt)
            left = proj_mms_left()
            if left > 0 and jt >= 2:
                quota = min(2, max(0, -(-left // max(1, (NSLOT - g)))))
                ci = proj_cursor[0]
                need_this_pair = 0
                for k in range(ci, len(proj_chunks)):
                    if proj_chunks[k][1] <= p + 1:
                        need_this_pair += 8
                    else:
                        break
                if need_this_pair:
                    need_this_pair -= proj_cursor[1]
                    pair_slots_left = 64 - (g % 64)
                    quota = min(2, max(quota, -(-need_this_pair // pair_slots_left)))
                for _ in range(quota):
                    emit_proj_mm()
            if g >= 3:
                emit_ctx(g - 3)
            if 4 <= jt <= 11:
                emit_drain_step()
        # pipeline tail: last three ctx slots, then the final drain fanned
        # out over the 4 now-free PSUM banks.
        emit_ctx(NSLOT - 3)
        emit_ctx(NSLOT - 2)
        emit_ctx(NSLOT - 1)
        k = 0
        while drain_q:
            emit_drain_step(tag=("proj", "tp", "c0", "c1")[k % 4])
            k += 1


def _make_in_maps(hidden_states, attention_mask, Wq, bq, Wk, bk, Wv, bv):
    bf = ml_dtypes.bfloat16

    def wqk_tiled(W, sl):
        # W.T shard [D, O] -> [128, NP, 8, 128]: row p holds
        # W.T[dt*128+p, pair*128:(pair+1)*128] at [p, pair, dt, :]
        wt = W[sl, :].T.astype(bf)  # [D, O]
        return np.ascontiguousarray(
            wt.reshape(8, 128, NP, 128).transpose(1, 2, 0, 3)
        )

    def wv_tiled(W, sl):
        wt = W[sl, :].T.astype(bf)  # [D, O]
        return np.ascontiguousarray(wt.reshape(8, 128, O).transpose(1, 0, 2))

    in_maps = []
    for c in range(8):
        b, hg = divmod(c, 2)
        sl = slice(hg * O, (hg + 1) * O)
        in_maps.append(
            {
                "xt": np.ascontiguousarray(hidden_states[b].T.astype(bf)),
                "wqt": wqk_tiled(Wq, sl),
                "wkt": wqk_tiled(Wk, sl),
                "wvt": wv_tiled(Wv, sl),
                "bq": np.ascontiguousarray(bq[sl]),
                "bk": np.ascontiguousarray(bk[sl]),
                "bv": np.ascontiguousarray(bv[sl]),
                "mask": np.ascontiguousarray(attention_mask[b, 0, 0, :]),
            }
        )
    return in_maps


def _gather(results):
    out = np.empty((B, S, D), dtype=np.float32)
    for c in range(8):
        b, hg = divmod(c, 2)
        out[b, :, hg * O : (hg + 1) * O] = results[c]["out"]
    return out


def kernel(hidden_states, attention_mask, Wq, bq, Wk, bk, Wv, bv, **run_kwargs):
    global _NC_CACHE
    args = [hidden_states, attention_mask, Wq, bq, Wk, bk, Wv, bv]
    args = [np.asarray(a, dtype=np.float32) for a in args]
    if _NC_CACHE is None:
        _NC_CACHE = build_nc()
    in_maps = _make_in_maps(*args)
    res = run_bass_kernel_spmd(_NC_CACHE, in_maps, core_ids=list(range(8)), **run_kwargs)
    kernel.last_result = res
    return _gather(res.results)


# revision 18
# speedup vs baseline: 1.0969x; 1.0078x over previous
"""BERT self-attention (B=4, S=2048, D=1024, H=16) on 8 trn2 NeuronCores.

Sharding: core c -> (batch b = c//2, head-group hg = c%2, 8 heads each).
Each core computes out[b, :, hg*512:(hg+1)*512] independently; host
gathers. Inputs are pre-transposed AND pre-tiled on host so every DMA is
contiguous >=4KB per partition: xt = X.T [D,S]; weights are partition-
major ([128, dt, ...] with row p holding W.T[dt*128+p, cols]).

v5 design (all-bf16, fully software-pipelined, ACT(exp)-paced):
  - V-projection raced against the xt DMA: dt-outer over ALL 8 PSUM
    banks (2 passes x 8 s-tiles); pass drains interleaved into the last
    dt row so DVE never serializes the pass handoff.
  - QK projection: kt(p0)+qt(p0,c0) up front; every remaining chunk
    drip-fed into attention jt-slots (PE slack while ACT runs exp),
    paced so pair p+1's Q/K finish during pair p, skipping the first 2
    slots of each qc (boundary pressure).
  - Attention per (pair, qc): scores for 2 heads as concurrent
    row-group matmuls -> [128,2,512] PSUM; ONE exp per jt ([128,1024]
    ACT op, mask as bias); ctx accumulates in PSUM and LAGS TWO SLOTS
    (sp bufs=2, u bufs=3) so PE never head-of-line blocks on exp and
    the C-bank drain gets slack before ctx(start=True) reuses it.
  - Drain: C -> SBUF copy (split per head, overlapped with last ctx),
    then 8 (transpose, reciprocal, scale) steps in slots 4..11 of the
    next qc; one batched DMA out per (p,qc).  Final drain fans out over
    4 free PSUM banks.
PSUM: sp0(2) + sp1(2) + c0(1) + c1(1) + proj(1) + tp(1) = 8 banks.
"""

import ml_dtypes
import numpy as np

import concourse.bass as bass
import concourse.tile as tile
from concourse import bacc, mybir
from concourse.bass_utils import run_bass_kernel_spmd
from concourse.masks import make_identity

B, S, D, H = 4, 2048, 1024, 16
DH = 64
O = 512  # per-core output width (8 heads)
HL = 8  # local heads per core
NP = 4  # head pairs per core
ST = S // 128  # 16 s-tiles
QC = 4  # query quarters (512 queries each)
F32 = mybir.dt.float32
BF16 = mybir.dt.bfloat16
EXP = mybir.ActivationFunctionType.Exp

_NC_CACHE = None


def build_nc():
    nc = bacc.Bacc(
        "TRN2",
        target_bir_lowering=False,
        debug=False,
        enable_asserts=True,
        num_devices=8,
    )
    xt = nc.dram_tensor("xt", [D, S], BF16, kind="ExternalInput").ap()
    # partition-major pre-tiled weights (see _make_in_maps)
    wqt = nc.dram_tensor("wqt", [128, NP, 8, 128], BF16, kind="ExternalInput").ap()
    wkt = nc.dram_tensor("wkt", [128, NP, 8, 128], BF16, kind="ExternalInput").ap()
    wvt = nc.dram_tensor("wvt", [128, 8, O], BF16, kind="ExternalInput").ap()
    bq = nc.dram_tensor("bq", [O], F32, kind="ExternalInput").ap()
    bk = nc.dram_tensor("bk", [O], F32, kind="ExternalInput").ap()
    bv = nc.dram_tensor("bv", [O], F32, kind="ExternalInput").ap()
    mask = nc.dram_tensor("mask", [S], F32, kind="ExternalInput").ap()
    out = nc.dram_tensor("out", [S, O], F32, kind="ExternalOutput").ap()

    with tile.TileContext(nc) as tc:
        _emit(nc, tc, xt, wqt, wkt, wvt, bq, bk, bv, mask, out)
    nc.compile()
    return nc


def _emit(nc, tc, xt, wqt, wkt, wvt, bq, bk, bv, mask, out):
    with (
        tc.tile_pool(name="singles", bufs=1) as singles,
        tc.tile_pool(name="persist", bufs=1) as persist,
        tc.tile_pool(name="wpool", bufs=1) as wpool,
        tc.tile_pool(name="attn", bufs=1) as attn,
        tc.tile_pool(name="psum", bufs=1, space="PSUM") as psum,
    ):
        # persistent activations (all bf16)
        xts = [persist.tile([128, S], BF16, name=f"xts{dt}", tag=f"xts{dt}") for dt in range(8)]
        qts = [persist.tile([128, S], BF16, name=f"qt{p}", tag=f"qt{p}") for p in range(NP)]
        kts = [persist.tile([128, S], BF16, name=f"kt{p}", tag=f"kt{p}") for p in range(NP)]
        vaug = [
            persist.tile([128, HL, DH + 1], BF16, name=f"vaug{t}", tag=f"vaug{t}")
            for t in range(ST)
        ]

        # DMA order is the startup critical path: wv first (V proj races
        # the xt stream), then xt, then small/late-needed tensors, then
        # QK weights (first used ~35us in).
        wv = wpool.tile([128, 8, O], BF16, name="wv", tag="wv")
        nc.sync.dma_start(out=wv[:, 0:1, :], in_=wvt[:, 0:1, :])
        nc.sync.dma_start(out=xts[0], in_=xt[0:128, :])
        nc.sync.dma_start(out=wv[:, 1:4, :], in_=wvt[:, 1:4, :])
        bv_bc = singles.tile([128, HL, DH], F32)
        nc.sync.dma_start(
            out=bv_bc, in_=bass.AP(tensor=bv.tensor, offset=0, ap=[[0, 128], [1, O]])
        )
        nc.sync.dma_start(out=xts[1], in_=xt[128:256, :])
        nc.sync.dma_start(out=wv[:, 4:8, :], in_=wvt[:, 4:8, :])
        for dt in range(2, 8):
            nc.sync.dma_start(out=xts[dt], in_=xt[dt * 128 : (dt + 1) * 128, :])
        ident = singles.tile([128, 128], F32)
        make_identity(nc, ident)
        wk = wpool.tile([128, NP, 8, 128], BF16, name="wk", tag="wk")
        nc.sync.dma_start(out=wk, in_=wkt)
        wq = wpool.tile([128, NP, 8, 128], BF16, name="wq", tag="wq")
        nc.sync.dma_start(out=wq, in_=wqt)
        wsl = {"k": wk, "q": wq}
        bq_sb = singles.tile([128, NP], F32)
        nc.sync.dma_start(out=bq_sb, in_=bq.rearrange("(t p) -> p t", p=128))
        bk_sb = singles.tile([128, NP], F32)
        nc.sync.dma_start(out=bk_sb, in_=bk.rearrange("(t p) -> p t", p=128))
        mask_sb = singles.tile([128, ST], F32)
        nc.sync.dma_start(out=mask_sb, in_=mask.rearrange("(t p) -> p t", p=128))
        # Schraudolph fast-exp constants for the DVE-offloaded slots:
        # exp(0.125*s + m) ~= bitcast_f32(int32(s*SCH_A + (m*SCH_M + SCH_B)))
        # with SCH_B tuned for minimax relative error (~+-3.5%).
        SCH_A = 0.125 * 1.4426950408889634 * 8388608.0
        SCH_M = 1.4426950408889634 * 8388608.0
        SCH_B = 127.0 * 8388608.0 - 297795.0
        mb_sb = singles.tile([128, ST], F32)
        nc.vector.tensor_scalar(
            out=mb_sb, in0=mask_sb, scalar1=SCH_M, scalar2=SCH_B,
            op0=mybir.AluOpType.mult, op1=mybir.AluOpType.add,
        )

        # vaug ones-columns: DVE is idle now, do them all up front
        for st in range(ST):
            nc.vector.memset(vaug[st][:, :, DH : DH + 1], 1.0)

        # ---- V projection: 2 passes x 8 s-tiles over all 8 PSUM banks,
        # dt-outer so pass 1 consumes xt chunks as the DMA delivers them.
        # Drains interleave into the dt=7 row so the pass handoff never
        # serializes on DVE.
        def v_pass(sb):  # sb = base s-tile (0 or 8)
            t01 = psum.tile([128, 2, HL, DH], F32, name=f"psv{sb}a", tag="sp0", bufs=1)
            t23 = psum.tile([128, 2, HL, DH], F32, name=f"psv{sb}b", tag="sp1", bufs=1)
            singles_ = [
                psum.tile([128, HL, DH], F32, name=f"psv{sb}_{i}", tag=t, bufs=1)
                for i, t in enumerate(("c0", "c1", "proj", "tp"))
            ]
            dsts = [t01[:, 0], t01[:, 1], t23[:, 0], t23[:, 1]] + singles_
            for dt in range(8):
                for g in range(8):
                    st = sb + g
                    nc.tensor.matmul(
                        dsts[g],
                        xts[dt][:, st * 128 : (st + 1) * 128],
                        wv[:, dt, :],
                        start=(dt == 0),
                        stop=(dt == 7),
                    )
                    if dt == 7:
                        va = vaug[st]
                        nc.vector.tensor_add(va[:, :, 0:DH], dsts[g], bv_bc)

        v_pass(0)
        v_pass(8)

        # ---- QK projection machinery ----
        qk_tag = [0]

        def emit_qk_chunk(which, p, c):
            w = wsl[which]
            dst = {"k": kts, "q": qts}[which][p]
            bias_sb = {"k": bk_sb, "q": bq_sb}[which]
            tag = ("proj", "tp")[qk_tag[0] % 2]
            qk_tag[0] += 1
            ps = psum.tile([128, 512], F32, name=f"ps{which}{p}_{c}", tag=tag, bufs=1)
            for dt in range(8):
                nc.tensor.matmul(
                    ps,
                    w[:, p, dt, :],
                    xts[dt][:, c * 512 : (c + 1) * 512],
                    start=(dt == 0),
                    stop=(dt == 7),
                )
            nc.vector.tensor_scalar_add(
                dst[:, c * 512 : (c + 1) * 512], ps, bias_sb[:, p : p + 1]
            )

        # Upfront: ALL of pair 0's QK.  Upfront matmuls chain back-to-back
        # at ~216ns while a second in-slot drip matmul costs ~330ns, so
        # pair 0 should only ever host pair 1's chunks (1 drip/slot).
        emit_qk_chunk("k", 0, 0)
        emit_qk_chunk("q", 0, 0)
        for c in range(1, 4):
            emit_qk_chunk("k", 0, c)
            emit_qk_chunk("q", 0, c)

        # Remaining chunks drip-fed into attention slots, one per slot.
        proj_chunks = []
        for p in range(1, NP):
            for c in range(4):
                proj_chunks.append(("k", p, c))
            for c in range(4):
                proj_chunks.append(("q", p, c))
        proj_cursor = [0, 0]  # chunk index, dt index

        def proj_mms_left():
            ci, dt = proj_cursor
            return (len(proj_chunks) - ci) * 8 - dt

        def emit_proj_mm():
            ci, dt = proj_cursor
            if ci >= len(proj_chunks):
                return False
            which, p, c = proj_chunks[ci]
            if dt == 0:
                emit_proj_mm.ps = psum.tile(
                    [128, 512], F32, name=f"ps{which}{p}_{c}", tag="proj", bufs=1
                )
            nc.tensor.matmul(
                emit_proj_mm.ps,
                wsl[which][:, p, dt, :],
                xts[dt][:, c * 512 : (c + 1) * 512],
                start=(dt == 0),
                stop=(dt == 7),
            )
            if dt == 7:
                dst = {"k": kts, "q": qts}[which][p]
                bias_sb = {"k": bk_sb, "q": bq_sb}[which]
                nc.vector.tensor_scalar_add(
                    dst[:, c * 512 : (c + 1) * 512], emit_proj_mm.ps,
                    bias_sb[:, p : p + 1],
                )
                proj_cursor[0] += 1
                proj_cursor[1] = 0
            else:
                proj_cursor[1] += 1
            return True

        # ---- attention: global software pipeline over 256 (p,qc,jt)
        # slots.  Slot g: scores(g) -> exp(g) -> [proj drip] -> ctx(g-2)
        # -> [drain step in slots 4..11].
        steps = [(p, qc, jt) for p in range(NP) for qc in range(QC) for jt in range(ST)]
        hist = {}
        Cs = {}
        drain_q = []

        def emit_scores(g, p, qc, jt):
            base = qc * 512
            sp = psum.tile([128, 2, 512], F32, name=f"sp{g}", tag=f"sp{g % 2}", bufs=1)
            for x in range(2):
                hp = slice(x * 64, x * 64 + 64)
                nc.tensor.matmul(
                    sp[:, x, :],
                    kts[p][hp, jt * 128 : (jt + 1) * 128],
                    qts[p][hp, base : base + 512],
                    start=True,
                    stop=True,
                )
            u = attn.tile([128, 2, 512], BF16, name=f"u{g}", tag=f"u{g % 4}", bufs=1)
            if jt == 13:
                # offload this slot's exp to the (otherwise idle) DVE via
                # Schraudolph's bit-trick so ACT catches up to the PE pace.
                iu = attn.tile(
                    [128, 2, 512], mybir.dt.int32, name=f"iu{g}", tag=f"iu{g % 2}",
                    bufs=1,
                )
                nc.vector.tensor_scalar(
                    out=iu, in0=sp, scalar1=SCH_A, scalar2=mb_sb[:, jt : jt + 1],
                    op0=mybir.AluOpType.mult, op1=mybir.AluOpType.add,
                )
                nc.vector.tensor_copy(out=u, in_=iu[:, :, :].bitcast(F32))
            else:
                nc.scalar.activation(
                    u, sp, EXP, bias=mask_sb[:, jt : jt + 1], scale=0.125
                )
            hist[g] = (u, p, qc, jt)

        def emit_ctx(g):
            u, p, qc, jt = hist.pop(g)
            if jt == 0:
                Cs[p, qc] = [
                    psum.tile(
                        [DH + 1, 512], F32, name=f"c{x}_{p}_{qc}", tag=f"c{x}", bufs=1
                    )
                    for x in range(2)
                ]
            C = Cs[p, qc]
            last = jt == ST - 1
            csb = []
            for x in range(2):
                nc.tensor.matmul(
                    C[x],
                    vaug[jt][:, 2 * p + x, :],
                    u[:, x, :],
                    start=(jt == 0),
                    stop=last,
                )
                if last:
                    cs = attn.tile(
                        [DH + 1, 512], F32, name=f"csb{p}_{qc}_{x}", tag=f"csb{x}",
                        bufs=2,
                    )
                    nc.vector.tensor_copy(out=cs, in_=C[x])
                    csb.append(cs)
            if last:
                Cs.pop((p, qc))
                ot = attn.tile(
                    [128, 4, 2, DH], F32, name=f"ot{p}_{qc}", tag="ot", bufs=2
                )
                drain_q.append({"csb": csb, "ot": ot, "p": p, "qc": qc, "i": 0})

        def emit_drain_step(tag="tp"):
            if not drain_q:
                return
            dr = drain_q[0]
            i = dr["i"]
            it, x = i // 2, i % 2
            tp_ = psum.tile(
                [128, DH + 1], F32, name=f"tp{dr['p']}_{dr['qc']}_{i}", tag=tag, bufs=1
            )
            nc.tensor.transpose(
                tp_,
                dr["csb"][x][:, it * 128 : (it + 1) * 128],
                ident[0 : DH + 1, 0 : DH + 1],
            )
            rc = attn.tile(
                [128, 1], F32, name=f"rc{dr['p']}_{dr['qc']}_{i}", tag="rc", bufs=4
            )
            nc.vector.reciprocal(rc, tp_[:, DH : DH + 1])
            nc.vector.tensor_scalar_mul(dr["ot"][:, it, x, :], tp_[:, 0:DH], rc)
            dr["i"] += 1
            if dr["i"] == 8:
                p, qc = dr["p"], dr["qc"]
                base = qc * 512
                nc.sync.dma_start(
                    out=out[
                        base : base + 512, 2 * p * DH : (2 * p + 2) * DH
                    ].rearrange("(i p) c -> p i c", p=128),
                    in_=dr["ot"],
                )
                drain_q.pop(0)

        NSLOT = len(steps)
        for g, (p, qc, jt) in enumerate(steps):
            emit_scores(g, p, qc, jt)
            left = proj_mms_left()
            if left > 0 and jt >= 2:
                quota = min(2, max(0, -(-left // max(1, (NSLOT - g)))))
                ci = proj_cursor[0]
                need_this_pair = 0
                for k in range(ci, len(proj_chunks)):
                    if proj_chunks[k][1] <= p + 1:
                        need_this_pair += 8
                    else:
                        break
                if need_this_pair:
                    need_this_pair -= proj_cursor[1]
                    pair_slots_left = 64 - (g % 64)
                    quota = min(2, max(quota, -(-need_this_pair // pair_slots_left)))
                for _ in range(quota):
                    emit_proj_mm()
            if g >= 3:
                emit_ctx(g - 3)
            if 4 <= jt <= 11:
                emit_drain_step()
        # pipeline tail: last three ctx slots, then the final drain fanned
        # out over the 4 now-free PSUM banks.
        emit_ctx(NSLOT - 3)
        emit_ctx(NSLOT - 2)
        emit_ctx(NSLOT - 1)
        k = 0
        while drain_q:
            emit_drain_step(tag=("proj", "tp", "c0", "c1")[k % 4])
            k += 1


def _make_in_maps(hidden_states, attention_mask, Wq, bq, Wk, bk, Wv, bv):
    bf = ml_dtypes.bfloat16

    def wqk_tiled(W, sl):
        # W.T shard [D, O] -> [128, NP, 8, 128]: row p holds
        # W.T[dt*128+p, pair*128:(pair+1)*128] at [p, pair, dt, :]
        wt = W[sl, :].T.astype(bf)  # [D, O]
        return np.ascontiguousarray(
            wt.reshape(8, 128, NP, 128).transpose(1, 2, 0, 3)
        )

    def wv_tiled(W, sl):
        wt = W[sl, :].T.astype(bf)  # [D, O]
        return np.ascontiguousarray(wt.reshape(8, 128, O).transpose(1, 0, 2))

    in_maps = []
    for c in range(8):
        b, hg = divmod(c, 2)
        sl = slice(hg * O, (hg + 1) * O)
        in_maps.append(
            {
                "xt": np.ascontiguousarray(hidden_states[b].T.astype(bf)),
                "wqt": wqk_tiled(Wq, sl),
                "wkt": wqk_tiled(Wk, sl),
                "wvt": wv_tiled(Wv, sl),
                "bq": np.ascontiguousarray(bq[sl]),
                "bk": np.ascontiguousarray(bk[sl]),
                "bv": np.ascontiguousarray(bv[sl]),
                "mask": np.ascontiguousarray(attention_mask[b, 0, 0, :]),
            }
        )
    return in_maps


def _gather(results):
    out = np.empty((B, S, D), dtype=np.float32)
    for c in range(8):
        b, hg = divmod(c, 2)
        out[b, :, hg * O : (hg + 1) * O] = results[c]["out"]
    return out


def kernel(hidden_states, attention_mask, Wq, bq, Wk, bk, Wv, bv, **run_kwargs):
    global _NC_CACHE
    args = [hidden_states, attention_mask, Wq, bq, Wk, bk, Wv, bv]
    args = [np.asarray(a, dtype=np.float32) for a in args]
    if _NC_CACHE is None:
        _NC_CACHE = build_nc()
    in_maps = _make_in_maps(*args)
    res = run_bass_kernel_spmd(_NC_CACHE, in_maps, core_ids=list(range(8)), **run_kwargs)
    kernel.last_result = res
    return _gather(res.results)


# revision 20
# speedup vs baseline: 1.1089x; 1.0110x over previous
"""BERT self-attention (B=4, S=2048, D=1024, H=16) on 8 trn2 NeuronCores.

Sharding: core c -> (batch b = c//2, head-group hg = c%2, 8 heads each).
Each core computes out[b, :, hg*512:(hg+1)*512] independently; host
gathers. Inputs are pre-transposed AND pre-tiled on host so every DMA is
contiguous >=4KB per partition: xt = X.T [D,S]; weights are partition-
major ([128, dt, ...] with row p holding W.T[dt*128+p, cols]).

v5 design (all-bf16, fully software-pipelined, ACT(exp)-paced):
  - V-projection raced against the xt DMA: dt-outer over ALL 8 PSUM
    banks (2 passes x 8 s-tiles); pass drains interleaved into the last
    dt row so DVE never serializes the pass handoff.
  - QK projection: kt(p0)+qt(p0,c0) up front; every remaining chunk
    drip-fed into attention jt-slots (PE slack while ACT runs exp),
    paced so pair p+1's Q/K finish during pair p, skipping the first 2
    slots of each qc (boundary pressure).
  - Attention per (pair, qc): scores for 2 heads as concurrent
    row-group matmuls -> [128,2,512] PSUM; ONE exp per jt ([128,1024]
    ACT op, mask as bias); ctx accumulates in PSUM and LAGS TWO SLOTS
    (sp bufs=2, u bufs=3) so PE never head-of-line blocks on exp and
    the C-bank drain gets slack before ctx(start=True) reuses it.
  - Drain: C -> SBUF copy (split per head, overlapped with last ctx),
    then 8 (transpose, reciprocal, scale) steps in slots 4..11 of the
    next qc; one batched DMA out per (p,qc).  Final drain fans out over
    4 free PSUM banks.
PSUM: sp0(2) + sp1(2) + c0(1) + c1(1) + proj(1) + tp(1) = 8 banks.
"""

import ml_dtypes
import numpy as np

import concourse.bass as bass
import concourse.tile as tile
from concourse import bacc, mybir
from concourse.bass_utils import run_bass_kernel_spmd
from concourse.masks import make_identity

B, S, D, H = 4, 2048, 1024, 16
DH = 64
O = 512  # per-core output width (8 heads)
HL = 8  # local heads per core
NP = 4  # head pairs per core
ST = S // 128  # 16 s-tiles
QC = 4  # query quarters (512 queries each)
F32 = mybir.dt.float32
BF16 = mybir.dt.bfloat16
EXP = mybir.ActivationFunctionType.Exp

_NC_CACHE = None


def build_nc():
    nc = bacc.Bacc(
        "TRN2",
        target_bir_lowering=False,
        debug=False,
        enable_asserts=True,
        num_devices=8,
    )
    xt = nc.dram_tensor("xt", [D, S], BF16, kind="ExternalInput").ap()
    # partition-major pre-tiled weights (see _make_in_maps)
    wqt = nc.dram_tensor("wqt", [128, NP, 8, 128], BF16, kind="ExternalInput").ap()
    wkt = nc.dram_tensor("wkt", [128, NP, 8, 128], BF16, kind="ExternalInput").ap()
    wvt = nc.dram_tensor("wvt", [128, 8, O], BF16, kind="ExternalInput").ap()
    bq = nc.dram_tensor("bq", [O], F32, kind="ExternalInput").ap()
    bk = nc.dram_tensor("bk", [O], F32, kind="ExternalInput").ap()
    bv = nc.dram_tensor("bv", [O], F32, kind="ExternalInput").ap()
    mask = nc.dram_tensor("mask", [S], F32, kind="ExternalInput").ap()
    out = nc.dram_tensor("out", [S, O], F32, kind="ExternalOutput").ap()

    with tile.TileContext(nc) as tc:
        _emit(nc, tc, xt, wqt, wkt, wvt, bq, bk, bv, mask, out)
    nc.compile()
    return nc


def _emit(nc, tc, xt, wqt, wkt, wvt, bq, bk, bv, mask, out):
    with (
        tc.tile_pool(name="singles", bufs=1) as singles,
        tc.tile_pool(name="persist", bufs=1) as persist,
        tc.tile_pool(name="wpool", bufs=1) as wpool,
        tc.tile_pool(name="attn", bufs=1) as attn,
        tc.tile_pool(name="psum", bufs=1, space="PSUM") as psum,
    ):
        # persistent activations (all bf16)
        xts = [persist.tile([128, S], BF16, name=f"xts{dt}", tag=f"xts{dt}") for dt in range(8)]
        qts = [persist.tile([128, S], BF16, name=f"qt{p}", tag=f"qt{p}") for p in range(NP)]
        kts = [persist.tile([128, S], BF16, name=f"kt{p}", tag=f"kt{p}") for p in range(NP)]
        vaug = [
            persist.tile([128, HL, DH + 1], BF16, name=f"vaug{t}", tag=f"vaug{t}")
            for t in range(ST)
        ]

        # DMA order is the startup critical path: wv first (V proj races
        # the xt stream), then xt, then small/late-needed tensors, then
        # QK weights (first used ~35us in).
        wv = wpool.tile([128, 8, O], BF16, name="wv", tag="wv")
        nc.sync.dma_start(out=wv[:, 0:1, :], in_=wvt[:, 0:1, :])
        nc.sync.dma_start(out=xts[0], in_=xt[0:128, :])
        nc.sync.dma_start(out=wv[:, 1:4, :], in_=wvt[:, 1:4, :])
        bv_bc = singles.tile([128, HL, DH], F32)
        nc.sync.dma_start(
            out=bv_bc, in_=bass.AP(tensor=bv.tensor, offset=0, ap=[[0, 128], [1, O]])
        )
        nc.sync.dma_start(out=xts[1], in_=xt[128:256, :])
        nc.sync.dma_start(out=wv[:, 4:8, :], in_=wvt[:, 4:8, :])
        for dt in range(2, 8):
            nc.sync.dma_start(out=xts[dt], in_=xt[dt * 128 : (dt + 1) * 128, :])
        ident = singles.tile([128, 128], F32)
        make_identity(nc, ident)
        wk = wpool.tile([128, NP, 8, 128], BF16, name="wk", tag="wk")
        nc.sync.dma_start(out=wk, in_=wkt)
        wq = wpool.tile([128, NP, 8, 128], BF16, name="wq", tag="wq")
        nc.sync.dma_start(out=wq, in_=wqt)
        wsl = {"k": wk, "q": wq}
        bq_sb = singles.tile([128, NP], F32)
        nc.sync.dma_start(out=bq_sb, in_=bq.rearrange("(t p) -> p t", p=128))
        bk_sb = singles.tile([128, NP], F32)
        nc.sync.dma_start(out=bk_sb, in_=bk.rearrange("(t p) -> p t", p=128))
        mask_sb = singles.tile([128, ST], F32)
        nc.sync.dma_start(out=mask_sb, in_=mask.rearrange("(t p) -> p t", p=128))
        # Schraudolph fast-exp constants for the DVE-offloaded slots:
        # exp(0.125*s + m) ~= bitcast_f32(int32(s*SCH_A + (m*SCH_M + SCH_B)))
        # with SCH_B tuned for minimax relative error (~+-3.5%).
        SCH_A = 0.125 * 1.4426950408889634 * 8388608.0
        SCH_M = 1.4426950408889634 * 8388608.0
        SCH_B = 127.0 * 8388608.0 - 297795.0
        mb_sb = singles.tile([128, ST], F32)
        nc.vector.tensor_scalar(
            out=mb_sb, in0=mask_sb, scalar1=SCH_M, scalar2=SCH_B,
            op0=mybir.AluOpType.mult, op1=mybir.AluOpType.add,
        )

        # vaug ones-columns: DVE is idle now, do them all up front
        for st in range(ST):
            nc.vector.memset(vaug[st][:, :, DH : DH + 1], 1.0)

        # ---- V projection: 2 passes x 8 s-tiles over all 8 PSUM banks,
        # dt-outer so pass 1 consumes xt chunks as the DMA delivers them.
        # Drains interleave into the dt=7 row so the pass handoff never
        # serializes on DVE.
        def v_pass(sb):  # sb = base s-tile (0 or 8)
            t01 = psum.tile([128, 2, HL, DH], F32, name=f"psv{sb}a", tag="sp0", bufs=1)
            t23 = psum.tile([128, 2, HL, DH], F32, name=f"psv{sb}b", tag="sp1", bufs=1)
            # proj/tp first: their drains gate the QK chunks that follow
            singles_ = [
                psum.tile([128, HL, DH], F32, name=f"psv{sb}_{i}", tag=t, bufs=1)
                for i, t in enumerate(("proj", "tp", "c0", "c1"))
            ]
            dsts = [t01[:, 0], t01[:, 1], t23[:, 0], t23[:, 1]] + singles_
            # proj/tp banks first in every row: their pass-1 drains unblock
            # pass 2's first matmuls, and their pass-2 drains unblock the
            # upfront QK chunks
            for dt in range(8):
                for g in (4, 5, 6, 7, 0, 1, 2, 3):
                    st = sb + g
                    nc.tensor.matmul(
                        dsts[g],
                        xts[dt][:, st * 128 : (st + 1) * 128],
                        wv[:, dt, :],
                        start=(dt == 0),
                        stop=(dt == 7),
                    )
                    if dt == 7:
                        va = vaug[st]
                        nc.vector.tensor_add(va[:, :, 0:DH], dsts[g], bv_bc)

        v_pass(0)
        v_pass(8)

        # ---- QK projection machinery ----
        qk_tag = [0]

        def emit_qk_chunk(which, p, c):
            w = wsl[which]
            dst = {"k": kts, "q": qts}[which][p]
            bias_sb = {"k": bk_sb, "q": bq_sb}[which]
            tag = ("proj", "tp")[qk_tag[0] % 2]
            qk_tag[0] += 1
            ps = psum.tile([128, 512], F32, name=f"ps{which}{p}_{c}", tag=tag, bufs=1)
            for dt in range(8):
                nc.tensor.matmul(
                    ps,
                    w[:, p, dt, :],
                    xts[dt][:, c * 512 : (c + 1) * 512],
                    start=(dt == 0),
                    stop=(dt == 7),
                )
            nc.vector.tensor_scalar_add(
                dst[:, c * 512 : (c + 1) * 512], ps, bias_sb[:, p : p + 1]
            )

        # Upfront: ALL of pair 0's QK.  Upfront matmuls chain back-to-back
        # at ~216ns while a second in-slot drip matmul costs ~330ns, so
        # pair 0 should only ever host pair 1's chunks (1 drip/slot).
        emit_qk_chunk("k", 0, 0)
        emit_qk_chunk("q", 0, 0)
        for c in range(1, 4):
            emit_qk_chunk("k", 0, c)
            emit_qk_chunk("q", 0, c)

        # Remaining chunks drip-fed into attention slots, one per slot.
        proj_chunks = []
        for p in range(1, NP):
            for c in range(4):
                proj_chunks.append(("k", p, c))
            for c in range(4):
                proj_chunks.append(("q", p, c))
        proj_cursor = [0, 0]  # chunk index, dt index

        def proj_mms_left():
            ci, dt = proj_cursor
            return (len(proj_chunks) - ci) * 8 - dt

        def emit_proj_mm():
            ci, dt = proj_cursor
            if ci >= len(proj_chunks):
                return False
            which, p, c = proj_chunks[ci]
            if dt == 0:
                emit_proj_mm.ps = psum.tile(
                    [128, 512], F32, name=f"ps{which}{p}_{c}", tag="proj", bufs=1
                )
            nc.tensor.matmul(
                emit_proj_mm.ps,
                wsl[which][:, p, dt, :],
                xts[dt][:, c * 512 : (c + 1) * 512],
                start=(dt == 0),
                stop=(dt == 7),
            )
            if dt == 7:
                dst = {"k": kts, "q": qts}[which][p]
                bias_sb = {"k": bk_sb, "q": bq_sb}[which]
                nc.vector.tensor_scalar_add(
                    dst[:, c * 512 : (c + 1) * 512], emit_proj_mm.ps,
                    bias_sb[:, p : p + 1],
                )
                proj_cursor[0] += 1
                proj_cursor[1] = 0
            else:
                proj_cursor[1] += 1
            return True

        # ---- attention: global software pipeline over 256 (p,qc,jt)
        # slots.  Slot g: scores(g) -> exp(g) -> [proj drip] -> ctx(g-2)
        # -> [drain step in slots 4..11].
        steps = [(p, qc, jt) for p in range(NP) for qc in range(QC) for jt in range(ST)]
        hist = {}
        Cs = {}
        drain_q = []

        def emit_scores(g, p, qc, jt):
            base = qc * 512
            sp = psum.tile([128, 2, 512], F32, name=f"sp{g}", tag=f"sp{g % 2}", bufs=1)
            for x in range(2):
                hp = slice(x * 64, x * 64 + 64)
                nc.tensor.matmul(
                    sp[:, x, :],
                    kts[p][hp, jt * 128 : (jt + 1) * 128],
                    qts[p][hp, base : base + 512],
                    start=True,
                    stop=True,
                )
            u = attn.tile([128, 2, 512], BF16, name=f"u{g}", tag=f"u{g % 4}", bufs=1)
            if jt == 13:
                # offload this slot's exp to the (otherwise idle) DVE via
                # Schraudolph's bit-trick so ACT catches up to the PE pace.
                iu = attn.tile(
                    [128, 2, 512], mybir.dt.int32, name=f"iu{g}", tag=f"iu{g % 2}",
                    bufs=1,
                )
                nc.vector.tensor_scalar(
                    out=iu, in0=sp, scalar1=SCH_A, scalar2=mb_sb[:, jt : jt + 1],
                    op0=mybir.AluOpType.mult, op1=mybir.AluOpType.add,
                )
                nc.vector.tensor_copy(out=u, in_=iu[:, :, :].bitcast(F32))
            else:
                nc.scalar.activation(
                    u, sp, EXP, bias=mask_sb[:, jt : jt + 1], scale=0.125
                )
            hist[g] = (u, p, qc, jt)

        def emit_ctx(g):
            u, p, qc, jt = hist.pop(g)
            if jt == 0:
                Cs[p, qc] = [
                    psum.tile(
                        [DH + 1, 512], F32, name=f"c{x}_{p}_{qc}", tag=f"c{x}", bufs=1
                    )
                    for x in range(2)
                ]
            C = Cs[p, qc]
            last = jt == ST - 1
            csb = []
            for x in range(2):
                nc.tensor.matmul(
                    C[x],
                    vaug[jt][:, 2 * p + x, :],
                    u[:, x, :],
                    start=(jt == 0),
                    stop=last,
                )
                if last:
                    cs = attn.tile(
                        [DH + 1, 512], F32, name=f"csb{p}_{qc}_{x}", tag=f"csb{x}",
                        bufs=2,
                    )
                    nc.vector.tensor_copy(out=cs, in_=C[x])
                    csb.append(cs)
            if last:
                Cs.pop((p, qc))
                ot = attn.tile(
                    [128, 4, 2, DH], F32, name=f"ot{p}_{qc}", tag="ot", bufs=2
                )
                drain_q.append({"csb": csb, "ot": ot, "p": p, "qc": qc, "i": 0})

        def emit_drain_step(tag="tp"):
            if not drain_q:
                return
            dr = drain_q[0]
            i = dr["i"]
            it, x = i // 2, i % 2
            tp_ = psum.tile(
                [128, DH + 1], F32, name=f"tp{dr['p']}_{dr['qc']}_{i}", tag=tag, bufs=1
            )
            nc.tensor.transpose(
                tp_,
                dr["csb"][x][:, it * 128 : (it + 1) * 128],
                ident[0 : DH + 1, 0 : DH + 1],
            )
            rc = attn.tile(
                [128, 1], F32, name=f"rc{dr['p']}_{dr['qc']}_{i}", tag="rc", bufs=4
            )
            nc.vector.reciprocal(rc, tp_[:, DH : DH + 1])
            nc.vector.tensor_scalar_mul(dr["ot"][:, it, x, :], tp_[:, 0:DH], rc)
            dr["i"] += 1
            if dr["i"] == 8:
                p, qc = dr["p"], dr["qc"]
                base = qc * 512
                nc.sync.dma_start(
                    out=out[
                        base : base + 512, 2 * p * DH : (2 * p + 2) * DH
                    ].rearrange("(i p) c -> p i c", p=128),
                    in_=dr["ot"],
                )
                drain_q.pop(0)

        NSLOT = len(steps)
        for g, (p, qc, jt) in enumerate(steps):
            emit_scores(g, p, qc, jt)
            left = proj_mms_left()
            if left > 0 and jt >= 2:
                quota = min(2, max(0, -(-left // max(1, (NSLOT - g)))))
                ci = proj_cursor[0]
                need_this_pair = 0
                for k in range(ci, len(proj_chunks)):
                    if proj_chunks[k][1] <= p + 1:
                        need_this_pair += 8
                    else:
                        break
                if need_this_pair:
                    need_this_pair -= proj_cursor[1]
                    pair_slots_left = 64 - (g % 64)
                    quota = min(2, max(quota, -(-need_this_pair // pair_slots_left)))
                for _ in range(quota):
                    emit_proj_mm()
            if g >= 3:
                emit_ctx(g - 3)
            if 4 <= jt <= 11:
                emit_drain_step()
        # pipeline tail: last three ctx slots, then the final drain fanned
        # out over the 4 now-free PSUM banks.
        emit_ctx(NSLOT - 3)
        emit_ctx(NSLOT - 2)
        emit_ctx(NSLOT - 1)
        k = 0
        while drain_q:
            emit_drain_step(tag=("proj", "tp", "c0", "c1")[k % 4])
            k += 1


def _make_in_maps(hidden_states, attention_mask, Wq, bq, Wk, bk, Wv, bv):
    bf = ml_dtypes.bfloat16

    def wqk_tiled(W, sl):
        # W.T shard [D, O] -> [128, NP, 8, 128]: row p holds
        # W.T[dt*128+p, pair*128:(pair+1)*128] at [p, pair, dt, :]
        wt = W[sl, :].T.astype(bf)  # [D, O]
        return np.ascontiguousarray(
            wt.reshape(8, 128, NP, 128).transpose(1, 2, 0, 3)
        )

    def wv_tiled(W, sl):
        wt = W[sl, :].T.astype(bf)  # [D, O]
        return np.ascontiguousarray(wt.reshape(8, 128, O).transpose(1, 0, 2))

    in_maps = []
    for c in range(8):
        b, hg = divmod(c, 2)
        sl = slice(hg * O, (hg + 1) * O)
        in_maps.append(
            {
                "xt": np.ascontiguousarray(hidden_states[b].T.astype(bf)),
                "wqt": wqk_tiled(Wq, sl),
                "wkt": wqk_tiled(Wk, sl),
                "wvt": wv_tiled(Wv, sl),
                "bq": np.ascontiguousarray(bq[sl]),
                "bk": np.ascontiguousarray(bk[sl]),
                "bv": np.ascontiguousarray(bv[sl]),
                "mask": np.ascontiguousarray(attention_mask[b, 0, 0, :]),
            }
        )
    return in_maps


def _gather(results):
    out = np.empty((B, S, D), dtype=np.float32)
    for c in range(8):
        b, hg = divmod(c, 2)
        out[b, :, hg * O : (hg + 1) * O] = results[c]["out"]
    return out


def kernel(hidden_states, attention_mask, Wq, bq, Wk, bk, Wv, bv, **run_kwargs):
    global _NC_CACHE
    args = [hidden_states, attention_mask, Wq, bq, Wk, bk, Wv, bv]
    args = [np.asarray(a, dtype=np.float32) for a in args]
    if _NC_CACHE is None:
        _NC_CACHE = build_nc()
    in_maps = _make_in_maps(*args)
    res = run_bass_kernel_spmd(_NC_CACHE, in_maps, core_ids=list(range(8)), **run_kwargs)
    kernel.last_result = res
    return _gather(res.results)
